# revision 1
# baseline (speedup 1.0000x reference)
"""DeepseekV3 MoE (E=16, K=4, H=1024, I=512, shared 2x) on 8 trn2 NeuronCores.

Expert-parallel: 2 routed experts per core (host gathers each expert's tokens),
shared expert + residual data-parallel over 512-token slices. Host does the
gate (fp32 numpy, reference-exact), the token all-to-all (gather/scatter), the
cw combine-weight fold and the residual add; all matmuls/activations run
on-device in bf16 with fp32 accumulation.

Device formulation keeps tokens on the matmul MOVING dim throughout
(weights/acts stationary), so activations come out pre-transposed and no PE
transposes are needed; the down-proj consumes act^T directly as stationary.
All inputs are host-pre-permuted to partition-major [128, ...] layouts so
every DMA is 128 long contiguous descriptors, sliced on stage boundaries.
"""

import os
import sys
import types
import numpy as np
import ml_dtypes

import concourse.bass as bass
import concourse.mybir as mybir
import concourse.tile as tile
from concourse import bacc
from concourse.bass_utils import run_bass_kernel_spmd

BF16 = mybir.dt.bfloat16
F32 = mybir.dt.float32
NP_BF16 = ml_dtypes.bfloat16

E, K, NG, TG = 16, 4, 4, 2
SCALE = 2.5
H, I, SH_I = 1024, 512, 1024
B, S = 2, 2048
N = B * S
NCORES = 8
EPC = E // NCORES          # experts per core = 2
NSH = N // NCORES          # shared-expert tokens per core = 512
HC = H // 128              # 8 h-chunks
IC = I // 128              # 4 i-chunks (routed)
SIC = SH_I // 128          # 8 i-chunks (shared)
GRAN = 64                  # per-expert token-capacity granularity


def _gate_cw(xf: np.ndarray, gate_w: np.ndarray, gate_bias: np.ndarray) -> np.ndarray:
    """Reference-exact MoE gate in numpy fp32. Returns cw [N, E]."""
    logits = xf @ gate_w.T
    scores = 1.0 / (1.0 + np.exp(-logits))
    sfc = scores + gate_bias
    epg = E // NG
    grp = sfc.reshape(N, NG, epg)
    top2 = np.sort(grp, axis=-1)[:, :, -2:].sum(-1)
    gidx = np.argsort(-top2, axis=1, kind="stable")[:, :TG]
    gmask = np.zeros((N, NG), bool)
    np.put_along_axis(gmask, gidx, True, axis=1)
    emask = np.repeat(gmask, epg, axis=1)
    masked = np.where(emask, sfc, -np.inf)
    topk_idx = np.argsort(-masked, axis=1, kind="stable")[:, :K]
    topk_w = np.take_along_axis(scores, topk_idx, axis=1)
    topk_w = topk_w / (topk_w.sum(-1, keepdims=True) + 1e-20)
    topk_w = topk_w * SCALE
    cw = np.zeros((N, E), np.float32)
    np.put_along_axis(cw, topk_idx, topk_w.astype(np.float32), axis=1)
    return cw


def _blocks(cap: int) -> list[int]:
    """Split cap into near-equal GRAN-multiple G/U token blocks of <=512."""
    nb = -(-cap // 512)
    base = (cap // nb) // GRAN * GRAN
    sizes = [base] * nb
    rem = cap - base * nb
    i = 0
    while rem > 0:
        sizes[i] += GRAN
        rem -= GRAN
        i = (i + 1) % nb
    return sizes


_BUILD_CACHE: dict[tuple, object] = {}


def _build(cea: int, ceb: int):
    """Build + compile the per-core SPMD Tile program."""
    key = (cea, ceb)
    if key in _BUILD_CACHE:
        return _BUILD_CACHE[key]
    m = cea + ceb
    eblocks = [_blocks(cea), _blocks(ceb)]
    xgw = HC * m

    nc = bacc.Bacc("TRN2", target_bir_lowering=False, debug=False,
                   num_devices=NCORES)
    xg_t = nc.dram_tensor("xg_t", [128, xgw], BF16, kind="ExternalInput").ap()
    wg_t = nc.dram_tensor("wg_t", [128, EPC, IC, HC, 128], BF16,
                          kind="ExternalInput").ap()
    wu_t = nc.dram_tensor("wu_t", [128, EPC, IC, HC, 128], BF16,
                          kind="ExternalInput").ap()
    wd_t = nc.dram_tensor("wd_t", [128, EPC, IC, H], BF16,
                          kind="ExternalInput").ap()
    xs_t = nc.dram_tensor("xs_t", [128, HC, NSH], BF16,
                          kind="ExternalInput").ap()
    wsg_t = nc.dram_tensor("wsg_t", [128, SIC, HC, 128], BF16,
                           kind="ExternalInput").ap()
    wsu_t = nc.dram_tensor("wsu_t", [128, SIC, HC, 128], BF16,
                           kind="ExternalInput").ap()
    wsd_t = nc.dram_tensor("wsd_t", [128, SIC, H], BF16,
                           kind="ExternalInput").ap()
    yg = nc.dram_tensor("yg", [m, H], BF16, kind="ExternalOutput").ap()
    ysh = nc.dram_tensor("ysh", [NSH, H], BF16, kind="ExternalOutput").ap()

    SILU = mybir.ActivationFunctionType.Silu

    with tile.TileContext(nc) as tc:
        with (
            tc.tile_pool(name="const", bufs=1) as const,
            tc.tile_pool(name="sb_s", bufs=4) as sb_s,
            tc.tile_pool(name="sb_a", bufs=3) as sb_a,
            tc.tile_pool(name="sb_y", bufs=3) as sb_y,
            tc.tile_pool(name="ps_gu", bufs=4, space=bass.MemorySpace.PSUM) as ps_gu,
            tc.tile_pool(name="ps_y", bufs=4, space=bass.MemorySpace.PSUM) as ps_y,
        ):
            # ---- resident SBUF loads; every dma is 128 contiguous
            # descriptor lines, ordered so the shared expert starts early
            # and i-slices arrive in j-loop consumption order.
            xs_sb = const.tile([128, HC, NSH], BF16, tag="xs")
            wsg_sb = const.tile([128, SIC, HC, 128], BF16, tag="wsg")
            wsu_sb = const.tile([128, SIC, HC, 128], BF16, tag="wsu")
            # first stage's data, finest-grained first so the opening G
            # accumulation starts as soon as its first h-chunk lands
            nc.sync.dma_start(xs_sb[:, 0], xs_t[:, 0])
            nc.sync.dma_start(wsg_sb[:, 0], wsg_t[:, 0])
            nc.sync.dma_start(wsu_sb[:, 0], wsu_t[:, 0])
            for c in range(1, HC):
                nc.sync.dma_start(xs_sb[:, c], xs_t[:, c])
            for j in range(1, SIC):
                nc.sync.dma_start(wsg_sb[:, j], wsg_t[:, j])
                nc.sync.dma_start(wsu_sb[:, j], wsu_t[:, j])
            wsd_sb = const.tile([128, SIC, H], BF16, tag="wsd")
            wg_sb = const.tile([128, EPC, IC, HC, 128], BF16, tag="wg")
            wu_sb = const.tile([128, EPC, IC, HC, 128], BF16, tag="wu")
            wd_sb = const.tile([128, EPC, IC, H], BF16, tag="wd")
            nc.sync.dma_start(wg_sb[:, 0], wg_t[:, 0])
            nc.sync.dma_start(wu_sb[:, 0], wu_t[:, 0])
            xgb = []          # per routed block: (e, b0, blk, sbuf tile)
            off = 0
            base = 0
            for e in range(EPC):
                b0 = base
                for blk in eblocks[e]:
                    t_ = const.tile([128, HC, blk], BF16, tag=f"xgb{len(xgb)}")
                    xgb.append((e, b0, blk, t_))
                    b0 += blk
                base += (cea, ceb)[e]
            for k, (e, b0, blk, t_) in enumerate(xgb):
                nc.sync.dma_start(
                    t_[:], xg_t[:, off:off + HC * blk].rearrange(
                        "p (c w) -> p c w", c=HC))
                off += HC * blk
                if k == 0:
                    nc.sync.dma_start(wsd_sb[:], wsd_t[:])
                if k == len(eblocks[0]) - 1:
                    nc.sync.dma_start(wd_sb[:, 0], wd_t[:, 0])
                    nc.sync.dma_start(wg_sb[:, 1], wg_t[:, 1])
                    nc.sync.dma_start(wu_sb[:, 1], wu_t[:, 1])
            nc.sync.dma_start(wd_sb[:, 1], wd_t[:, 1])

            # ---- stage bodies ----
            def gu_shared():
                """G/U + act for all shared-expert tokens, i-slice-major so
                each j needs only its own wsg/wsu slice (streams behind a
                single DMA queue without stalling)."""
                act = sb_a.tile([128, SIC, NSH], BF16, tag="act")
                for j in range(SIC):
                    g = ps_gu.tile([128, NSH], F32, tag="gu")
                    u = ps_gu.tile([128, NSH], F32, tag="gu")
                    for c in range(HC):
                        nc.tensor.matmul(g[:], wsg_sb[:, j, c], xs_sb[:, c],
                                         start=(c == 0), stop=(c == HC - 1))
                    for c in range(HC):
                        nc.tensor.matmul(u[:], wsu_sb[:, j, c], xs_sb[:, c],
                                         start=(c == 0), stop=(c == HC - 1))
                    s = sb_s.tile([128, NSH], BF16, tag="sig")
                    nc.scalar.activation(s[:], g[:], SILU)
                    nc.vector.tensor_mul(act[:, j, :], s[:], u[:])
                return act

            def down_shared(act):
                for t in range(NSH // 128):
                    r0 = t * 128
                    ts = slice(t * 128, (t + 1) * 128)
                    y0 = ps_y.tile([128, 512], F32, tag="y_ps")
                    for j in range(SIC):
                        nc.tensor.matmul(y0[:], act[:, j, ts], wsd_sb[:, j, :512],
                                         start=(j == 0), stop=(j == SIC - 1))
                    y1 = ps_y.tile([128, 512], F32, tag="y_ps")
                    for j in range(SIC):
                        nc.tensor.matmul(y1[:], act[:, j, ts], wsd_sb[:, j, 512:],
                                         start=(j == 0), stop=(j == SIC - 1))
                    y_sb = sb_y.tile([128, H], BF16, tag="y")
                    nc.scalar.copy(y_sb[:, :512], y0[:])
                    nc.vector.tensor_copy(y_sb[:, 512:], y1[:])
                    nc.sync.dma_start(ysh[r0:r0 + 128, :], y_sb[:])

            def gu_routed(e, blk, xg_b):
                """G/U + act for one gathered-token block of expert-slot e."""
                act = sb_a.tile([128, IC, blk], BF16, tag="act")
                for j in range(IC):
                    g = ps_gu.tile([128, blk], F32, tag="gu")
                    u = ps_gu.tile([128, blk], F32, tag="gu")
                    for c in range(HC):
                        nc.tensor.matmul(g[:], wg_sb[:, e, j, c], xg_b[:, c],
                                         start=(c == 0), stop=(c == HC - 1))
                    for c in range(HC):
                        nc.tensor.matmul(u[:], wu_sb[:, e, j, c], xg_b[:, c],
                                         start=(c == 0), stop=(c == HC - 1))
                    s = sb_s.tile([128, blk], BF16, tag="sig")
                    nc.scalar.activation(s[:], g[:], SILU)
                    nc.vector.tensor_mul(act[:, j, :], s[:], u[:])
                return act

            def down_routed(e, b0, blk, act):
                for t0 in range(0, blk, 128):
                    tw = min(128, blk - t0)
                    y0 = ps_y.tile([128, 512], F32, tag="y_ps")
                    for j in range(IC):
                        nc.tensor.matmul(y0[:tw, :], act[:, j, t0:t0 + tw],
                                         wd_sb[:, e, j, :512],
                                         start=(j == 0), stop=(j == IC - 1))
                    y1 = ps_y.tile([128, 512], F32, tag="y_ps")
                    for j in range(IC):
                        nc.tensor.matmul(y1[:tw, :], act[:, j, t0:t0 + tw],
                                         wd_sb[:, e, j, 512:],
                                         start=(j == 0), stop=(j == IC - 1))
                    y_sb = sb_y.tile([128, H], BF16, tag="y")
                    nc.scalar.copy(y_sb[:tw, :512], y0[:tw, :])
                    nc.vector.tensor_copy(y_sb[:tw, 512:], y1[:tw, :])
                    nc.sync.dma_start(yg[b0 + t0:b0 + t0 + tw, :], y_sb[:tw, :])

            # ---- 2-stage software pipeline: emit stage k+1's G/U before
            # stage k's down-proj so the PE has fill work during the DVE
            # act latency of stage k+1.
            work = [(gu_shared, lambda a: down_shared(a))]
            # smallest routed block last: smaller final copy + output DMA
            xgb_sched = sorted(xgb, key=lambda r: (r[2] <= min(x[2] for x in xgb),))
            for e, b0, blk, t_ in xgb_sched:
                work.append((lambda e=e, blk=blk, t_=t_: gu_routed(e, blk, t_),
                             lambda a, e=e, b0=b0, blk=blk:
                             down_routed(e, b0, blk, a)))
            pend = None
            for gu_f, dn_f in work:
                act = gu_f()
                if pend is not None:
                    pend[1](pend[0])
                pend = (act, dn_f)
            pend[1](pend[0])

    nc.compile()
    _BUILD_CACHE[key] = nc
    return nc


def _pp_stat(wt: np.ndarray) -> np.ndarray:
    """[H_, I_] (contraction-major) -> [128, I_/128, H_/128, 128] stationary."""
    Hd, Id = wt.shape
    return np.ascontiguousarray(
        wt.reshape(Hd // 128, 128, Id // 128, 128).transpose(1, 2, 0, 3))


def _pp_mov(mt: np.ndarray) -> np.ndarray:
    """[K_, F] (contraction-major) -> [128, K_/128, F] moving."""
    Kd, Fd = mt.shape
    return np.ascontiguousarray(mt.reshape(Kd // 128, 128, Fd).transpose(1, 0, 2))


def _prepare(inputs: dict, caps, pairs, idx: list[np.ndarray]):
    """Build per-core input maps. idx[e] = token indices routed to expert e."""
    xf = np.asarray(inputs["hidden_states"], np.float32).reshape(N, H)
    xt_bf = np.ascontiguousarray(xf.T).astype(NP_BF16)        # [H, N]
    wg = np.asarray(inputs["Wg"], np.float32)
    wu = np.asarray(inputs["Wu"], np.float32)
    wd = np.asarray(inputs["Wd"], np.float32)
    wsg = np.asarray(inputs["Ws_g"], np.float32)
    wsu = np.asarray(inputs["Ws_u"], np.float32)
    wsd = np.asarray(inputs["Ws_d"], np.float32)
    eblocks = [_blocks(caps[0]), _blocks(caps[1])]

    wsg_p = _pp_stat(wsg.T.astype(NP_BF16))
    wsu_p = _pp_stat(wsu.T.astype(NP_BF16))
    wsd_p = _pp_mov(wsd.T.astype(NP_BF16))
    wg_p = {e: _pp_stat(wg[e].T.astype(NP_BF16)) for e in range(E)}
    wu_p = {e: _pp_stat(wu[e].T.astype(NP_BF16)) for e in range(E)}
    wd_p = {e: _pp_mov(wd[e].T.astype(NP_BF16)) for e in range(E)}

    in_maps = []
    for core in range(NCORES):
        es = pairs[core]
        segs = []
        for j, e in enumerate(es):
            ne = len(idx[e])
            xe = np.zeros((H, caps[j]), NP_BF16)
            xe[:, :ne] = xt_bf[:, idx[e]]
            b0 = 0
            for blk in eblocks[j]:
                segs.append(_pp_mov(xe[:, b0:b0 + blk]).reshape(128, -1))
                b0 += blk
        xg_p = np.ascontiguousarray(np.concatenate(segs, axis=1))
        sl = slice(core * NSH, (core + 1) * NSH)
        xs_p = _pp_mov(xt_bf[:, sl])
        in_maps.append({
            "xg_t": xg_p,
            "wg_t": np.ascontiguousarray(np.stack([wg_p[e] for e in es], 1)),
            "wu_t": np.ascontiguousarray(np.stack([wu_p[e] for e in es], 1)),
            "wd_t": np.ascontiguousarray(np.stack([wd_p[e] for e in es], 1)),
            "xs_t": xs_p,
            "wsg_t": wsg_p,
            "wsu_t": wsu_p,
            "wsd_t": wsd_p,
        })
    return in_maps


def _combine(results, caps, pairs, cw: np.ndarray, xf: np.ndarray,
             idx: list[np.ndarray]) -> np.ndarray:
    out = xf.copy()
    bases = [0, caps[0]]
    for core in range(NCORES):
        out[core * NSH:(core + 1) * NSH] += np.asarray(
            results[core]["ysh"], np.float32)
    for core in range(NCORES):
        ygr = np.asarray(results[core]["yg"], np.float32)
        for j, e in enumerate(pairs[core]):
            ne = len(idx[e])
            out[idx[e]] += ygr[bases[j]:bases[j] + ne] * cw[idx[e], e][:, None]
    return out.reshape(B, S, H)


def _route(inputs: dict):
    xf = np.asarray(inputs["hidden_states"], np.float32).reshape(N, H)
    cw = _gate_cw(xf, np.asarray(inputs["gate_w"], np.float32),
                  np.asarray(inputs["gate_bias"], np.float32))
    idx = [np.nonzero(cw[:, e])[0] for e in range(E)]
    loads = np.array([len(i) for i in idx])
    order = np.argsort(-loads, kind="stable")
    bigs, smalls = order[:NCORES], order[NCORES:][::-1]
    pairs = [(int(a), int(b)) for a, b in zip(bigs, smalls)]
    cea = max(256, -(-int(loads[bigs].max()) // GRAN) * GRAN)
    ceb = max(256, -(-int(loads[smalls].max()) // GRAN) * GRAN)
    return cw, xf, idx, (cea, ceb), pairs


def _run(inputs: dict, trace: bool = False, tmpdir: str | None = None):
    cw, xf, idx, caps, pairs = _route(inputs)
    nc = _build(*caps)
    in_maps = _prepare(inputs, caps, pairs, idx)
    res = run_bass_kernel_spmd(nc, in_maps, list(range(NCORES)),
                               trace=trace, tmpdir=tmpdir)
    return _combine(res.results, caps, pairs, cw, xf, idx), res


def kernel(**inputs) -> np.ndarray:
    out, _ = _run(inputs, trace=False)
    return out


def _install_prof_shim():
    """Make run_bass_kernel_spmd(trace=True) work under axon in this image."""
    if "antenv.axon_hooks" in sys.modules:
        return
    try:
        from trn_agent_boot.trn_boot import _ntff_profile_via_ctypes
        hook = _ntff_profile_via_ctypes("/opt/axon/libaxon_pjrt.so")
    except Exception:
        hook = None
    mod = types.ModuleType("antenv.axon_hooks")
    mod.get_axon_ntff_profile_hook = lambda: hook
    mod.set_axon_ntff_profile_hook = lambda h: None
    sys.modules["antenv.axon_hooks"] = mod
    import concourse.bass_utils as bu
    bu.upload_artifacts = lambda tmpdir: tmpdir


def kernel_traced(tmpdir=None, all_cores=False, **inputs):
    """Returns (output, BassKernelResults with exec_time_ns)."""
    _install_prof_shim()
    if all_cores:
        os.environ["BASS_PERFETTO_PROFILE_ALL_CORES"] = "1"
    out, res = _run(inputs, trace=True, tmpdir=tmpdir)
    return out, res



# revision 5
# speedup vs baseline: 1.0049x; 1.0049x over previous
"""DeepseekV3 MoE (E=16, K=4, H=1024, I=512, shared 2x) on 8 trn2 NeuronCores.

Expert-parallel: 2 routed experts per core (host gathers each expert's tokens),
shared expert + residual data-parallel over 512-token slices. Host does the
gate (fp32 numpy, reference-exact), the token all-to-all (gather/scatter), the
cw combine-weight fold and the residual add; all matmuls/activations run
on-device in bf16 with fp32 accumulation.

Device formulation keeps tokens on the matmul MOVING dim throughout
(weights/acts stationary), so activations come out pre-transposed and no PE
transposes are needed; the down-proj consumes act^T directly as stationary.
All inputs are host-pre-permuted to partition-major [128, ...] layouts so
every DMA is 128 long contiguous descriptors, sliced on stage boundaries.
"""

import os
import sys
import types
import numpy as np
import ml_dtypes

import concourse.bass as bass
import concourse.mybir as mybir
import concourse.tile as tile
from concourse import bacc
from concourse.bass_utils import run_bass_kernel_spmd

BF16 = mybir.dt.bfloat16
F32 = mybir.dt.float32
NP_BF16 = ml_dtypes.bfloat16

E, K, NG, TG = 16, 4, 4, 2
SCALE = 2.5
H, I, SH_I = 1024, 512, 1024
B, S = 2, 2048
N = B * S
NCORES = 8
EPC = E // NCORES          # experts per core = 2
NSH = N // NCORES          # shared-expert tokens per core = 512
HC = H // 128              # 8 h-chunks
IC = I // 128              # 4 i-chunks (routed)
SIC = SH_I // 128          # 8 i-chunks (shared)
GRAN = 64                  # per-expert token-capacity granularity


def _gate_cw(xf: np.ndarray, gate_w: np.ndarray, gate_bias: np.ndarray) -> np.ndarray:
    """Reference-exact MoE gate in numpy fp32. Returns cw [N, E]."""
    logits = xf @ gate_w.T
    scores = 1.0 / (1.0 + np.exp(-logits))
    sfc = scores + gate_bias
    epg = E // NG
    grp = sfc.reshape(N, NG, epg)
    top2 = np.sort(grp, axis=-1)[:, :, -2:].sum(-1)
    gidx = np.argsort(-top2, axis=1, kind="stable")[:, :TG]
    gmask = np.zeros((N, NG), bool)
    np.put_along_axis(gmask, gidx, True, axis=1)
    emask = np.repeat(gmask, epg, axis=1)
    masked = np.where(emask, sfc, -np.inf)
    topk_idx = np.argsort(-masked, axis=1, kind="stable")[:, :K]
    topk_w = np.take_along_axis(scores, topk_idx, axis=1)
    topk_w = topk_w / (topk_w.sum(-1, keepdims=True) + 1e-20)
    topk_w = topk_w * SCALE
    cw = np.zeros((N, E), np.float32)
    np.put_along_axis(cw, topk_idx, topk_w.astype(np.float32), axis=1)
    return cw


def _blocks(cap: int) -> list[int]:
    """Split cap into near-equal GRAN-multiple G/U token blocks of <=512."""
    nb = -(-cap // 512)
    base = (cap // nb) // GRAN * GRAN
    sizes = [base] * nb
    rem = cap - base * nb
    i = 0
    while rem > 0:
        sizes[i] += GRAN
        rem -= GRAN
        i = (i + 1) % nb
    return sizes


_BUILD_CACHE: dict[tuple, object] = {}


def _build(cea: int, ceb: int):
    """Build + compile the per-core SPMD Tile program."""
    key = (cea, ceb)
    if key in _BUILD_CACHE:
        return _BUILD_CACHE[key]
    m = cea + ceb
    eblocks = [_blocks(cea), _blocks(ceb)]
    xgw = HC * m

    nc = bacc.Bacc("TRN2", target_bir_lowering=False, debug=False,
                   num_devices=NCORES)
    xg_t = nc.dram_tensor("xg_t", [128, xgw], BF16, kind="ExternalInput").ap()
    wg_t = nc.dram_tensor("wg_t", [128, EPC, IC, HC, 128], BF16,
                          kind="ExternalInput").ap()
    wu_t = nc.dram_tensor("wu_t", [128, EPC, IC, HC, 128], BF16,
                          kind="ExternalInput").ap()
    wd_t = nc.dram_tensor("wd_t", [128, EPC, IC, H], BF16,
                          kind="ExternalInput").ap()
    xs_t = nc.dram_tensor("xs_t", [128, HC, NSH], BF16,
                          kind="ExternalInput").ap()
    wsg_t = nc.dram_tensor("wsg_t", [128, SIC, HC, 128], BF16,
                           kind="ExternalInput").ap()
    wsu_t = nc.dram_tensor("wsu_t", [128, SIC, HC, 128], BF16,
                           kind="ExternalInput").ap()
    wsd_t = nc.dram_tensor("wsd_t", [128, SIC, H], BF16,
                           kind="ExternalInput").ap()
    yg = nc.dram_tensor("yg", [m, H], BF16, kind="ExternalOutput").ap()
    ysh = nc.dram_tensor("ysh", [NSH, H], BF16, kind="ExternalOutput").ap()

    SILU = mybir.ActivationFunctionType.Silu

    with tile.TileContext(nc) as tc:
        with (
            tc.tile_pool(name="const", bufs=1) as const,
            tc.tile_pool(name="sb_s", bufs=4) as sb_s,
            tc.tile_pool(name="sb_a", bufs=3) as sb_a,
            tc.tile_pool(name="sb_y", bufs=3) as sb_y,
            tc.tile_pool(name="ps_gu", bufs=4, space=bass.MemorySpace.PSUM) as ps_gu,
            tc.tile_pool(name="ps_y", bufs=4, space=bass.MemorySpace.PSUM) as ps_y,
        ):
            # ---- resident SBUF loads. DMA-trigger instructions cost ~610ns
            # each on the issuing HWDGE queue, so inputs are few LARGE
            # transfers (fine-grained only for the first-consumed shared
            # tiles); all input triggers ride the SP queue while output
            # stores ride the Activation queue (down_* below) so outputs
            # never wait behind input triggers.
            xs_sb = const.tile([128, HC, NSH], BF16, tag="xs")
            wsg_sb = const.tile([128, SIC, HC, 128], BF16, tag="wsg")
            wsu_sb = const.tile([128, SIC, HC, 128], BF16, tag="wsu")
            wsd_sb = const.tile([128, SIC, H], BF16, tag="wsd")
            wg_sb = const.tile([128, EPC, IC, HC, 128], BF16, tag="wg")
            wu_sb = const.tile([128, EPC, IC, HC, 128], BF16, tag="wu")
            wd_sb = const.tile([128, EPC, IC, H], BF16, tag="wd")
            xgb = []          # per routed block: (e, b0, blk, sbuf tile)
            base = 0
            xgoff = []
            off = 0
            for e in range(EPC):
                b0 = base
                for blk in eblocks[e]:
                    t_ = const.tile([128, HC, blk], BF16, tag=f"xgb{len(xgb)}")
                    xgb.append((e, b0, blk, t_))
                    xgoff.append(off)
                    b0 += blk
                    off += HC * blk
                base += (cea, ceb)[e]
            # smallest routed block last: smaller final copy + output DMA
            sched_order = sorted(
                range(len(xgb)),
                key=lambda k: (xgb[k][2] <= min(x[2] for x in xgb),))

            def dma_xgb(k):
                e, b0, blk, t_ = xgb[k]
                nc.sync.dma_start(
                    t_[:], xg_t[:, xgoff[k]:xgoff[k] + HC * blk].rearrange(
                        "p (c w) -> p c w", c=HC))

            nc.sync.dma_start(xs_sb[:, 0:4], xs_t[:, 0:4])
            nc.sync.dma_start(wsg_sb[:, 0], wsg_t[:, 0])
            nc.sync.dma_start(wsu_sb[:, 0], wsu_t[:, 0])
            nc.sync.dma_start(xs_sb[:, 4:8], xs_t[:, 4:8])
            nc.sync.dma_start(wsg_sb[:, 1:4], wsg_t[:, 1:4])
            nc.sync.dma_start(wsu_sb[:, 1:4], wsu_t[:, 1:4])
            nc.sync.dma_start(wsg_sb[:, 4:8], wsg_t[:, 4:8])
            nc.sync.dma_start(wsu_sb[:, 4:8], wsu_t[:, 4:8])
            nc.sync.dma_start(wsd_sb[:], wsd_t[:])
            dma_xgb(sched_order[0])
            nc.sync.dma_start(wg_sb[:], wg_t[:])
            nc.sync.dma_start(wu_sb[:], wu_t[:])
            if len(sched_order) > 1:
                dma_xgb(sched_order[1])
            nc.sync.dma_start(wd_sb[:], wd_t[:])
            for k in sched_order[2:]:
                dma_xgb(k)

            # ---- stage bodies ----
            def gu_shared():
                """G/U + act for all shared-expert tokens, i-slice-major so
                each j needs only its own wsg/wsu slice (streams behind a
                single DMA queue without stalling)."""
                act = sb_a.tile([128, SIC, NSH], BF16, tag="act")
                for j in range(SIC):
                    g = ps_gu.tile([128, NSH], F32, tag="gu")
                    u = ps_gu.tile([128, NSH], F32, tag="gu")
                    for c in range(HC):
                        nc.tensor.matmul(g[:], wsg_sb[:, j, c], xs_sb[:, c],
                                         start=(c == 0), stop=(c == HC - 1))
                    for c in range(HC):
                        nc.tensor.matmul(u[:], wsu_sb[:, j, c], xs_sb[:, c],
                                         start=(c == 0), stop=(c == HC - 1))
                    s = sb_s.tile([128, NSH], BF16, tag="sig")
                    nc.scalar.activation(s[:], g[:], SILU)
                    nc.vector.tensor_mul(act[:, j, :], s[:], u[:])
                return act

            def down_shared(act):
                for t in range(NSH // 128):
                    r0 = t * 128
                    ts = slice(t * 128, (t + 1) * 128)
                    y0 = ps_y.tile([128, 512], F32, tag="y_ps")
                    for j in range(SIC):
                        nc.tensor.matmul(y0[:], act[:, j, ts], wsd_sb[:, j, :512],
                                         start=(j == 0), stop=(j == SIC - 1))
                    y1 = ps_y.tile([128, 512], F32, tag="y_ps")
                    for j in range(SIC):
                        nc.tensor.matmul(y1[:], act[:, j, ts], wsd_sb[:, j, 512:],
                                         start=(j == 0), stop=(j == SIC - 1))
                    y_sb = sb_y.tile([128, H], BF16, tag="y")
                    nc.scalar.copy(y_sb[:, :512], y0[:])
                    nc.vector.tensor_copy(y_sb[:, 512:], y1[:])
                    nc.scalar.dma_start(ysh[r0:r0 + 128, :], y_sb[:])

            def gu_routed(e, blk, xg_b):
                """G/U + act for one gathered-token block of expert-slot e."""
                act = sb_a.tile([128, IC, blk], BF16, tag="act")
                for j in range(IC):
                    g = ps_gu.tile([128, blk], F32, tag="gu")
                    u = ps_gu.tile([128, blk], F32, tag="gu")
                    for c in range(HC):
                        nc.tensor.matmul(g[:], wg_sb[:, e, j, c], xg_b[:, c],
                                         start=(c == 0), stop=(c == HC - 1))
                    for c in range(HC):
                        nc.tensor.matmul(u[:], wu_sb[:, e, j, c], xg_b[:, c],
                                         start=(c == 0), stop=(c == HC - 1))
                    s = sb_s.tile([128, blk], BF16, tag="sig")
                    nc.scalar.activation(s[:], g[:], SILU)
                    nc.vector.tensor_mul(act[:, j, :], s[:], u[:])
                return act

            def down_routed(e, b0, blk, act, last=False):
                for t0 in range(0, blk, 128):
                    tw = min(128, blk - t0)
                    y0 = ps_y.tile([128, 512], F32, tag="y_ps")
                    for j in range(IC):
                        nc.tensor.matmul(y0[:tw, :], act[:, j, t0:t0 + tw],
                                         wd_sb[:, e, j, :512],
                                         start=(j == 0), stop=(j == IC - 1))
                    y1 = ps_y.tile([128, 512], F32, tag="y_ps")
                    for j in range(IC):
                        nc.tensor.matmul(y1[:tw, :], act[:, j, t0:t0 + tw],
                                         wd_sb[:, e, j, 512:],
                                         start=(j == 0), stop=(j == IC - 1))
                    y_sb = sb_y.tile([128, H], BF16, tag="y")
                    r = slice(b0 + t0, b0 + t0 + tw)
                    if last and t0 + 128 >= blk:
                        # final tile of the whole kernel: split the store so
                        # the first half DMAs while the second half copies
                        nc.scalar.copy(y_sb[:tw, :512], y0[:tw, :])
                        nc.scalar.dma_start(yg[r, :512], y_sb[:tw, :512])
                        nc.vector.tensor_copy(y_sb[:tw, 512:], y1[:tw, :])
                        nc.scalar.dma_start(yg[r, 512:], y_sb[:tw, 512:])
                    else:
                        nc.scalar.copy(y_sb[:tw, :512], y0[:tw, :])
                        nc.vector.tensor_copy(y_sb[:tw, 512:], y1[:tw, :])
                        nc.scalar.dma_start(yg[r, :], y_sb[:tw, :])

            # ---- 2-stage software pipeline: emit stage k+1's G/U before
            # stage k's down-proj so the PE has fill work during the DVE
            # act latency of stage k+1.
            work = [(gu_shared, lambda a: down_shared(a))]
            for i, k in enumerate(sched_order):
                e, b0, blk, t_ = xgb[k]
                last = i == len(sched_order) - 1
                work.append((lambda e=e, blk=blk, t_=t_: gu_routed(e, blk, t_),
                             lambda a, e=e, b0=b0, blk=blk, last=last:
                             down_routed(e, b0, blk, a, last)))
            pend = None
            for gu_f, dn_f in work:
                act = gu_f()
                if pend is not None:
                    pend[1](pend[0])
                pend = (act, dn_f)
            pend[1](pend[0])

    nc.compile()
    _BUILD_CACHE[key] = nc
    return nc


def _pp_stat(wt: np.ndarray) -> np.ndarray:
    """[H_, I_] (contraction-major) -> [128, I_/128, H_/128, 128] stationary."""
    Hd, Id = wt.shape
    return np.ascontiguousarray(
        wt.reshape(Hd // 128, 128, Id // 128, 128).transpose(1, 2, 0, 3))


def _pp_mov(mt: np.ndarray) -> np.ndarray:
    """[K_, F] (contraction-major) -> [128, K_/128, F] moving."""
    Kd, Fd = mt.shape
    return np.ascontiguousarray(mt.reshape(Kd // 128, 128, Fd).transpose(1, 0, 2))


def _prepare(inputs: dict, caps, pairs, idx: list[np.ndarray]):
    """Build per-core input maps. idx[e] = token indices routed to expert e."""
    xf = np.asarray(inputs["hidden_states"], np.float32).reshape(N, H)
    xt_bf = np.ascontiguousarray(xf.T).astype(NP_BF16)        # [H, N]
    wg = np.asarray(inputs["Wg"], np.float32)
    wu = np.asarray(inputs["Wu"], np.float32)
    wd = np.asarray(inputs["Wd"], np.float32)
    wsg = np.asarray(inputs["Ws_g"], np.float32)
    wsu = np.asarray(inputs["Ws_u"], np.float32)
    wsd = np.asarray(inputs["Ws_d"], np.float32)
    eblocks = [_blocks(caps[0]), _blocks(caps[1])]

    wsg_p = _pp_stat(wsg.T.astype(NP_BF16))
    wsu_p = _pp_stat(wsu.T.astype(NP_BF16))
    wsd_p = _pp_mov(wsd.T.astype(NP_BF16))
    wg_p = {e: _pp_stat(wg[e].T.astype(NP_BF16)) for e in range(E)}
    wu_p = {e: _pp_stat(wu[e].T.astype(NP_BF16)) for e in range(E)}
    wd_p = {e: _pp_mov(wd[e].T.astype(NP_BF16)) for e in range(E)}

    in_maps = []
    for core in range(NCORES):
        es = pairs[core]
        segs = []
        for j, e in enumerate(es):
            ne = len(idx[e])
            xe = np.zeros((H, caps[j]), NP_BF16)
            xe[:, :ne] = xt_bf[:, idx[e]]
            b0 = 0
            for blk in eblocks[j]:
                segs.append(_pp_mov(xe[:, b0:b0 + blk]).reshape(128, -1))
                b0 += blk
        xg_p = np.ascontiguousarray(np.concatenate(segs, axis=1))
        sl = slice(core * NSH, (core + 1) * NSH)
        xs_p = _pp_mov(xt_bf[:, sl])
        in_maps.append({
            "xg_t": xg_p,
            "wg_t": np.ascontiguousarray(np.stack([wg_p[e] for e in es], 1)),
            "wu_t": np.ascontiguousarray(np.stack([wu_p[e] for e in es], 1)),
            "wd_t": np.ascontiguousarray(np.stack([wd_p[e] for e in es], 1)),
            "xs_t": xs_p,
            "wsg_t": wsg_p,
            "wsu_t": wsu_p,
            "wsd_t": wsd_p,
        })
    return in_maps


def _combine(results, caps, pairs, cw: np.ndarray, xf: np.ndarray,
             idx: list[np.ndarray]) -> np.ndarray:
    out = xf.copy()
    bases = [0, caps[0]]
    for core in range(NCORES):
        out[core * NSH:(core + 1) * NSH] += np.asarray(
            results[core]["ysh"], np.float32)
    for core in range(NCORES):
        ygr = np.asarray(results[core]["yg"], np.float32)
        for j, e in enumerate(pairs[core]):
            ne = len(idx[e])
            out[idx[e]] += ygr[bases[j]:bases[j] + ne] * cw[idx[e], e][:, None]
    return out.reshape(B, S, H)


def _route(inputs: dict):
    xf = np.asarray(inputs["hidden_states"], np.float32).reshape(N, H)
    cw = _gate_cw(xf, np.asarray(inputs["gate_w"], np.float32),
                  np.asarray(inputs["gate_bias"], np.float32))
    idx = [np.nonzero(cw[:, e])[0] for e in range(E)]
    loads = np.array([len(i) for i in idx])
    order = np.argsort(-loads, kind="stable")
    bigs, smalls = order[:NCORES], order[NCORES:][::-1]
    pairs = [(int(a), int(b)) for a, b in zip(bigs, smalls)]
    cea = max(256, -(-int(loads[bigs].max()) // GRAN) * GRAN)
    ceb = max(256, -(-int(loads[smalls].max()) // GRAN) * GRAN)
    return cw, xf, idx, (cea, ceb), pairs


def _run(inputs: dict, trace: bool = False, tmpdir: str | None = None):
    cw, xf, idx, caps, pairs = _route(inputs)
    nc = _build(*caps)
    in_maps = _prepare(inputs, caps, pairs, idx)
    res = run_bass_kernel_spmd(nc, in_maps, list(range(NCORES)),
                               trace=trace, tmpdir=tmpdir)
    return _combine(res.results, caps, pairs, cw, xf, idx), res


def kernel(**inputs) -> np.ndarray:
    out, _ = _run(inputs, trace=False)
    return out


def _install_prof_shim():
    """Make run_bass_kernel_spmd(trace=True) work under axon in this image."""
    if "antenv.axon_hooks" in sys.modules:
        return
    try:
        from trn_agent_boot.trn_boot import _ntff_profile_via_ctypes
        hook = _ntff_profile_via_ctypes("/opt/axon/libaxon_pjrt.so")
    except Exception:
        hook = None
    mod = types.ModuleType("antenv.axon_hooks")
    mod.get_axon_ntff_profile_hook = lambda: hook
    mod.set_axon_ntff_profile_hook = lambda h: None
    sys.modules["antenv.axon_hooks"] = mod
    import concourse.bass_utils as bu
    bu.upload_artifacts = lambda tmpdir: tmpdir


def kernel_traced(tmpdir=None, all_cores=False, **inputs):
    """Returns (output, BassKernelResults with exec_time_ns)."""
    _install_prof_shim()
    if all_cores:
        os.environ["BASS_PERFETTO_PROFILE_ALL_CORES"] = "1"
    out, res = _run(inputs, trace=True, tmpdir=tmpdir)
    return out, res

